# revision 2
# baseline (speedup 1.0000x reference)
"""Trainium2 Bass kernel for the bidirectional RNN language model.

Model (see problem reference): for a [L=128, B=32] int token grid,
  - forward + backward tanh-RNN (HID=20) over EMB=80 embeddings (VOCAB=32000)
  - per position: logits = [h_fwd[i], h_bwd[i+1]] @ h2o   -> [*, 32000]
  - output log_softmax(logits)  ->  [128, 32, 32000] f32  (512 MB)

Strategy: data-parallel over batch across 8 NeuronCores (4 batch columns
per core), no collectives. The 64 MB/core output write (~185 us at DMA
roofline) is the hard floor; everything else is scheduled to minimize
time-to-first-output-byte and keep the output stream gap-free:
  - recurrence starts at ~6 us: only the first embedding-gather pair
    gates step 0; the other gathers/transposes pipeline into the early
    steps (PE-transpose -> DVE copy straight into the operand buffer),
  - combined fwd+bwd recurrence: step tau = one K=112 matmul + one tanh,
  - at step 78 the recurrence PAUSES and tile 0's softmax-normalizer
    pass (32x [128,1024] exp chunks, ACT-serial ~34 us) runs
    immediately; output streaming starts at ~95 us,
  - position tiles are symmetric pairs {48-79},{32-47,80-95},
    {16-31,96-111},{0-15,112-127}: ready at steps 78/94/110/126, so each
    later tile's 16 recurrence steps + exp pass hide under the previous
    tile's 45.5 us output window,
  - exp partial sums are split across engines (ACT accum_out, Pool
    dummy-copy accum, DVE reduce) to fit ACT's per-window budget,
  - ln(sum) is computed WITHOUT the Ln activation (exponent-bit trick +
    one exp-based Newton step, abs err < 5e-4) so the whole kernel uses
    one activation-table set {Exp,Tanh,Copy,Identity} - no 1.3 us
    table reloads between tanh/exp phases,
  - pass 2 recomputes logit chunks (f32r matmul) and subtracts ln(sum)
    on DVE (ACT shares on the last tile) into staging; output groups
    ramp 1K/1K/2K/2K/4K... columns so the first bytes go out early.
Cost-model exec: ~280 us/core; DMA saturated from ~95 us on.
"""

import numpy as np

import concourse.bacc as bacc
import concourse.tile as tile
from concourse import bass, mybir
from concourse.bass_utils import run_bass_kernel_spmd
from concourse.masks import make_identity

L = 128
B = 32
V = 32000
EMB = 80
HID = 20
KDIM = EMB + HID          # 100
# Device-side contraction layout: hidden rows at partitions 0:20, zero pad
# 20:32 (compute-engine APs must start 32-aligned), embeddings at 32:112.
EOFF = 32
KP = EOFF + EMB           # 112
H2 = 2 * HID              # 40
NCORES = 8
BL = B // NCORES          # 4 batch columns per core
R = L * BL                # 512 output rows per core
NT = 4                    # position tiles of 128 rows (32 positions)

CH = 1024                 # vocab chunk per pass-1 PSUM tile (2 banks)
NFULL = V // CH           # 31 full chunks
REM = V - NFULL * CH      # 256
NVC = NFULL + 1           # 32 chunks
P2W = 512                 # pass-2 chunk width (1 PSUM bank)
NP2 = (V + P2W - 1) // P2W  # 63 pass-2 chunks (last = 256)
# Output staging groups per tile, in pass-2 chunk counts: ramp up so the
# first bytes hit HBM quickly after the normalizer lands.
GROUP_CHUNKS = [2, 2, 4, 4, 8, 8, 8, 8, 8, 8, 3]
assert sum(GROUP_CHUNKS) == NP2
SGW = 4096                # max staging width (16 KB/partition)

# Partial-sum routing (which engine reduces each pass-1 chunk).
PHASED_POOL = 14          # tile-0 serial phase: Pool takes c0..13
PHASED_DVE = 12           # ... DVE takes c14..25, ACT accum c26..31
WIN_POOL = 20             # windows: Pool takes c0..19, ACT accum the rest

# ln-approx constants: ln(x) ~= K1*float(bits(x)) - K2, |err| <= 0.0299,
# then one Newton step y += x*exp(-y) - 1 brings |err| < 5e-4.
LN_K1 = 8.262958405176314e-08   # ln2 / 2^23
LN_K2 = 87.99984328235631       # 127*ln2 - 0.02985

# Emission-order deadlines (us since window start) used to merge the
# per-window instruction streams; only relative order matters.
DL_REC = 0.644            # recurrence step period
DL_P2 = 0.658             # DVE subtract pace per 512 chunk
DL_P1_START = 10.9        # first exp after the 16-step tanh batch
DL_P1 = 1.03              # ACT exp pace per 1024 chunk

F32 = mybir.dt.float32
F32R = mybir.dt.float32r
I32 = mybir.dt.int32
AF = mybir.ActivationFunctionType
ALU = mybir.AluOpType
AXL = mybir.AxisListType

_CACHE = {}

# Optional extra kwargs for run_bass_kernel_spmd (used by test harness for
# tracing); harmless defaults for grading.
RUN_KWARGS = {}
LAST_RESULTS = None

# Symmetric position-tile pairs: tile pt = positions [a, a+16) u [b, b+16),
# ready after recurrence step max(b+15, 127-a) = 78/94/110/126.
PTS = [(48, 64), (32, 80), (16, 96), (0, 112)]


def _build():
    nc = bacc.Bacc("TRN2", debug=False, num_devices=NCORES)

    # idx rows 0..511: tokens in (position, batch) row-major order;
    # rows 512..1023: same with positions reversed (backward-chain gather).
    idx_d = nc.dram_tensor("idx", [2 * R, 1], I32, kind="ExternalInput")
    we_d = nc.dram_tensor("we", [V, EMB], F32, kind="ExternalInput")
    i2h_d = nc.dram_tensor("i2h", [KP, HID], F32, kind="ExternalInput")
    # float32r: PE streams fp32 at full rate with tf32-like operand
    # truncation - ~2e-4 relative effect on logits, far inside tolerance.
    h2o_d = nc.dram_tensor("h2o", [H2, V], F32R, kind="ExternalInput")
    biasc_d = nc.dram_tensor("biasc", [HID, 1], F32, kind="ExternalInput")
    h0r_d = nc.dram_tensor("h0r", [HID, 2 * BL], F32, kind="ExternalInput")
    out_d = nc.dram_tensor("out", [R, V], F32, kind="ExternalOutput")

    with tile.TileContext(nc) as tc:
        with (
            tc.tile_pool(name="const", bufs=1) as const,
            tc.tile_pool(name="hbuf", bufs=1) as hbuf,
            tc.tile_pool(name="gat", bufs=2) as gat,
            tc.tile_pool(name="stat", bufs=1) as stat,
            tc.tile_pool(name="stage", bufs=3) as stage,
        ):
            ident = const.tile([128, 128], F32)
            make_identity(nc, ident[:])
            # Warm the single activation table ({Exp,Tanh,Copy,Identity}
            # all live in exp_and_others) before the recurrence starts.
            warm = const.tile([1, 1], F32)
            nc.vector.memset(warm[:], 0.0)
            nc.scalar.activation(out=warm[:], in_=warm[:], func=AF.Exp)

            # One strided DMA loads all eight gather index columns (4 fwd +
            # 4 bwd): idx8[p, k] = idx[128k + p].
            idx8 = const.tile([128, 2 * NT], I32)
            nc.sync.dma_start(
                out=idx8[:],
                in_=bass.AP(tensor=idx_d, offset=0, ap=[[1, 128], [128, 2 * NT]]),
            )
            i2h_sb = const.tile([KP, HID], F32)
            nc.sync.dma_start(out=i2h_sb[:], in_=i2h_d[:, :])
            biasc = const.tile([HID, 1], F32)
            nc.sync.dma_start(out=biasc[:], in_=biasc_d[:, :])
            pool_scr = const.tile([128, CH], F32)

            # Combined recurrence operand buffers: step tau's block (8 cols)
            # at tile tau//32, local cols 8*(tau%32): [fwd tau | bwd 127-tau].
            # Rows 0:20 = hidden inputs ([hiddenf[tau] | hiddenb[128-tau]]),
            # rows 20:32 zero pad, rows 32:112 = embedding^T.
            rhsC = [
                hbuf.tile([KP, 256], F32, name=f"rhsC{k}", tag=f"rhsC{k}")
                for k in range(NT)
            ]
            for k in range(NT):
                nc.vector.memset(rhsC[k][HID:EOFF, :], 0.0)
            nc.sync.dma_start(out=rhsC[0][0:HID, 0:8], in_=h0r_d[:, :])

            h2o_sb = const.tile([H2, V], F32R)
            for q in range(4):
                nc.sync.dma_start(
                    out=h2o_sb[:, q * (V // 4) : (q + 1) * (V // 4)],
                    in_=h2o_d[:, q * (V // 4) : (q + 1) * (V // 4)],
                )

            # Embedding gathers, all queued upfront on the gpsimd (SWDGE)
            # queue; transposes/copies pipeline into the recurrence below.
            embG = []
            for k in range(NT):
                for half, icol in ((0, k), (1, NT + k)):
                    g = gat.tile([128, EMB], F32, tag="embG")
                    nc.gpsimd.indirect_dma_start(
                        out=g[:],
                        out_offset=None,
                        in_=we_d[:, :],
                        in_offset=bass.IndirectOffsetOnAxis(
                            ap=idx8[:, icol : icol + 1], axis=0
                        ),
                    )
                    embG.append((k, half, g))

            hcatT = [
                hbuf.tile([H2, 128], F32, name=f"hcatT{k}", tag=f"hcatT{k}")
                for k in range(NT)
            ]
            sparts = [
                stat.tile([128, NVC], F32, name=f"sparts{k}", tag=f"sparts{k}")
                for k in range(NT)
            ]
            logs = [
                stat.tile([128, 1], F32, name=f"logs{k}", tag=f"logs{k}")
                for k in range(NT)
            ]
            neg_log3 = stat.tile([128, 1], F32, name="nlog3", tag="nlog3")

            with (
                tc.tile_pool(name="rps", bufs=2, space="PSUM") as rps,
                tc.tile_pool(name="p1ps", bufs=2, space="PSUM") as p1ps,
                tc.tile_pool(name="p2ps", bufs=2, space="PSUM") as p2ps,
            ):

                def emit_tc(i):
                    # transpose gather i and scatter into rhsC via DVE
                    k, half, g = embG[i]
                    psT = p1ps.tile([128, CH], F32, tag="p1", name="p1t")
                    nc.tensor.transpose(
                        out=psT[0:EMB, 0:128], in_=g[:], identity=ident[:]
                    )
                    dst = rhsC[k][EOFF:, :].rearrange(
                        "p (b g) -> p b g", g=8
                    )[:, :, 4 * half : 4 * half + 4]
                    nc.vector.tensor_copy(out=dst, in_=psT[0:EMB, 0:128])

                def emit_rec(step):
                    k0, c0 = step // 32, 8 * (step % 32)
                    pc = rps.tile([HID, 2 * BL], F32, tag="rec")
                    nc.tensor.matmul(
                        out=pc[:],
                        lhsT=i2h_sb[:],
                        rhs=rhsC[k0][:, c0 : c0 + 8],
                        start=True,
                        stop=True,
                    )
                    t1 = step + 1
                    k1, c1 = t1 // 32, 8 * (t1 % 32)
                    nc.scalar.activation(
                        out=rhsC[k1][0:HID, c1 : c1 + 8],
                        in_=pc[:],
                        func=AF.Tanh,
                        bias=biasc[:],
                    )

                def emit_hcat(pt):
                    # assemble [40, 128] hidden-state lhsT on DVE (fwd rows
                    # ascending blocks, bwd rows descending blocks).
                    for s, p0 in enumerate(PTS[pt]):
                        d0 = 64 * s
                        kf, fc0 = p0 // 32, 8 * (p0 % 32)
                        tf = rhsC[kf]
                        src_f = bass.AP(
                            tensor=tf.tensor,
                            offset=tf.offset + fc0,
                            ap=[[tf.ap[0][0], HID], [8, 16], [1, 4]],
                        )
                        nc.vector.tensor_copy(
                            out=hcatT[pt][0:HID, d0 : d0 + 64], in_=src_f
                        )
                        b_hi = 127 - p0
                        kb, bc0 = b_hi // 32, 8 * (b_hi % 32) + 4
                        tb = rhsC[kb]
                        src_b = bass.AP(
                            tensor=tb.tensor,
                            offset=tb.offset + bc0,
                            ap=[[tb.ap[0][0], HID], [-8, 16], [1, 4]],
                        )
                        nc.vector.tensor_copy(
                            out=hcatT[pt][HID:, d0 : d0 + 64], in_=src_b
                        )

                def emit_p1(pt, vc, red):
                    # pass-1 chunk: logits to PSUM, exp in place, partial
                    # sum into sparts[pt][:, vc] via the routed engine.
                    v0 = vc * CH
                    w = CH if vc < NFULL else REM
                    p1t = p1ps.tile([128, CH], F32, tag="p1", name="p1t")
                    for m in range(0, w, 512):
                        mw = min(512, w - m)
                        nc.tensor.matmul(
                            out=p1t[:, m : m + mw],
                            lhsT=hcatT[pt][:].bitcast(F32R),
                            rhs=h2o_sb[:, v0 + m : v0 + m + mw],
                            start=True,
                            stop=True,
                        )
                    col = sparts[pt][:, vc : vc + 1]
                    if red == "act":
                        nc.scalar.activation(
                            out=p1t[:, :w], in_=p1t[:, :w], func=AF.Exp,
                            accum_out=col,
                        )
                    else:
                        nc.scalar.activation(
                            out=p1t[:, :w], in_=p1t[:, :w], func=AF.Exp
                        )
                        if red == "dve":
                            nc.vector.tensor_reduce(
                                out=col, in_=p1t[:, :w], axis=AXL.X, op=ALU.add
                            )
                        else:  # pool: dummy copy with accumulator
                            nc.gpsimd.tensor_scalar(
                                out=pool_scr[:, :w], in0=p1t[:, :w],
                                scalar1=1.0, scalar2=None, op0=ALU.mult,
                                accum_out=col,
                            )

                def emit_stats(pt):
                    # logs[pt] = ln(sum(sparts[pt])) without the Ln table:
                    # exponent-bit approx + one Newton step via Exp.
                    s_t = stat.tile([128, 1], F32, name=f"s{pt}", tag=f"s{pt}")
                    nc.vector.tensor_reduce(
                        out=s_t[:], in_=sparts[pt][:, :], axis=AXL.X, op=ALU.add
                    )
                    fb = stat.tile([128, 1], F32, name=f"fb{pt}", tag=f"fb{pt}")
                    nc.vector.tensor_copy(out=fb[:], in_=s_t[:].bitcast(I32))
                    y0 = stat.tile([128, 1], F32, name=f"y0{pt}", tag=f"y0{pt}")
                    nc.vector.tensor_scalar(
                        out=y0[:], in0=fb[:], scalar1=LN_K1, scalar2=-LN_K2,
                        op0=ALU.mult, op1=ALU.add,
                    )
                    te = stat.tile([128, 1], F32, name=f"te{pt}", tag=f"te{pt}")
                    nc.scalar.activation(
                        out=te[:], in_=y0[:], func=AF.Exp, scale=-1.0
                    )
                    u = stat.tile([128, 1], F32, name=f"u{pt}", tag=f"u{pt}")
                    nc.vector.tensor_mul(out=u[:], in0=s_t[:], in1=te[:])
                    nc.vector.scalar_tensor_tensor(
                        out=logs[pt][:], in0=y0[:], scalar=-1.0, in1=u[:],
                        op0=ALU.add, op1=ALU.add,
                    )
                    if pt == NT - 1:
                        nc.vector.tensor_scalar(
                            out=neg_log3[:], in0=logs[pt][:], scalar1=-1.0,
                            scalar2=None, op0=ALU.mult,
                        )

                # staging state for the output groups of the current tile
                st = {"stg": None, "off": 0, "g0": 0, "gi": 0}

                def emit_p2(pt, j, share_act=False):
                    v0 = j * P2W
                    w = P2W if j < NP2 - 1 else V - v0
                    p2t = p2ps.tile([128, P2W], F32, tag="p2", name="p2t")
                    nc.tensor.matmul(
                        out=p2t[:, :w],
                        lhsT=hcatT[pt][:].bitcast(F32R),
                        rhs=h2o_sb[:, v0 : v0 + w],
                        start=True,
                        stop=True,
                    )
                    if st["off"] == 0:
                        st["stg"] = stage.tile(
                            [128, SGW], F32, tag="stg", name="stg"
                        )
                        st["g0"] = v0
                    off = st["off"]
                    if share_act:
                        nc.scalar.activation(
                            out=st["stg"][:, off : off + w],
                            in_=p2t[:, :w],
                            func=AF.Identity,
                            bias=neg_log3[:],
                        )
                    else:
                        nc.vector.tensor_scalar(
                            out=st["stg"][:, off : off + w],
                            in0=p2t[:, :w],
                            scalar1=logs[pt][:],
                            scalar2=None,
                            op0=ALU.subtract,
                        )
                    st["off"] = off + w
                    # close the group when its chunk count is reached
                    gend = sum(GROUP_CHUNKS[: st["gi"] + 1])
                    if j + 1 == gend:
                        gw = st["off"]
                        r0a, r0b = 4 * PTS[pt][0], 4 * PTS[pt][1]
                        dst = bass.AP(
                            tensor=out_d,
                            offset=r0a * V + st["g0"],
                            ap=[[(r0b - r0a) * V, 2], [V, 64], [1, gw]],
                        )
                        nc.sync.dma_start(out=dst, in_=st["stg"][:, :gw])
                        st["off"] = 0
                        st["gi"] = (st["gi"] + 1) % len(GROUP_CHUNKS)

                # --- prefix: recurrence steps 0..78, gather chains woven in
                emit_tc(0)
                emit_tc(1)
                for step in range(79):
                    emit_rec(step)
                    if step in (8, 16, 24, 32, 40, 48):
                        emit_tc(2 + (step - 8) // 8)

                # --- tile 0 normalizer phase (recurrence paused)
                emit_hcat(0)
                for vc in range(NVC):
                    red = ("pool" if vc < PHASED_POOL
                           else "dve" if vc < PHASED_POOL + PHASED_DVE
                           else "act")
                    emit_p1(0, vc, red)
                emit_stats(0)

                # --- four output windows; window pt streams tile pt while
                # the next tile's recurrence tail + exp pass run under it.
                for pt in range(NT):
                    last = pt == NT - 1
                    ev = []
                    if not last:
                        for i in range(16):
                            ev.append((DL_REC * i, 0, "rec", 79 + 16 * pt + i))
                        ev.append((10.0, 0, "hcat", pt + 1))
                        for k in range(NVC):
                            ev.append(
                                (DL_P1_START + DL_P1 * k - 0.6, 1, "p1", k)
                            )
                    for j in range(NP2):
                        ev.append((max(DL_P2 * j - 0.4, 0.05), 2, "p2", j))
                    ev.sort(key=lambda e: (e[0], e[1]))
                    for _, _, kind, a in ev:
                        if kind == "rec":
                            emit_rec(a)
                        elif kind == "hcat":
                            emit_hcat(a)
                        elif kind == "p1":
                            red = "pool" if a < WIN_POOL else "act"
                            emit_p1(pt + 1, a, red)
                        else:
                            emit_p2(pt, a, share_act=last and (a % 2 == 1))
                    if not last:
                        emit_stats(pt + 1)

    nc.compile()
    return nc


def _get_nc():
    if "nc" not in _CACHE:
        _CACHE["nc"] = _build()
    return _CACHE["nc"]


def kernel(input, we, i2h, h2o, bias, h0):
    global LAST_RESULTS
    input = np.asarray(input)
    we = np.ascontiguousarray(np.asarray(we), dtype=np.float32)
    i2h = np.ascontiguousarray(np.asarray(i2h), dtype=np.float32)
    h2o = np.ascontiguousarray(np.asarray(h2o), dtype=np.float32)
    bias = np.asarray(bias, dtype=np.float32)
    h0 = np.asarray(h0, dtype=np.float32)

    biasc = np.ascontiguousarray(bias.reshape(1, HID).T)          # [20, 1]
    h0r = np.ascontiguousarray(
        np.repeat(h0.reshape(1, HID).T, 2 * BL, axis=1)           # [20, 8]
    )
    # Reorder i2h into the padded device contraction layout: hidden-state
    # weight rows first, zeros, then embedding weight rows.
    i2h_dev = np.zeros((KP, HID), dtype=np.float32)
    i2h_dev[0:HID] = i2h[EMB:]
    i2h_dev[EOFF:] = i2h[0:EMB]

    nc = _get_nc()
    in_maps = []
    for c in range(NCORES):
        tok = input[:, BL * c : BL * (c + 1)].astype(np.int32)    # [L, BL]
        idx = np.ascontiguousarray(
            np.concatenate([tok.reshape(R), tok[::-1].reshape(R)]).reshape(
                2 * R, 1
            )
        )
        in_maps.append(
            {
                "idx": idx,
                "we": we,
                "i2h": i2h_dev,
                "h2o": h2o,
                "biasc": biasc,
                "h0r": h0r,
            }
        )

    res = run_bass_kernel_spmd(
        nc, in_maps, core_ids=list(range(NCORES)), **RUN_KWARGS
    )
    LAST_RESULTS = res
    parts = [res.results[c]["out"].reshape(L, BL, V) for c in range(NCORES)]
    return np.concatenate(parts, axis=1)


# revision 11
# speedup vs baseline: 1.0944x; 1.0944x over previous
"""Trainium2 Bass kernel for the bidirectional RNN language model.

Model (see problem reference): for a [L=128, B=32] int token grid,
  - forward + backward tanh-RNN (HID=20) over EMB=80 embeddings (VOCAB=32000)
  - per position: logits = [h_fwd[i], h_bwd[i+1]] @ h2o   -> [*, 32000]
  - output log_softmax(logits)  ->  [128, 32, 32000] f32  (512 MB)

Strategy: data-parallel over batch across 8 NeuronCores (4 batch columns
per core), no collectives. The 64 MB/core output write (~185 us at DMA
roofline) is the hard floor; everything else is scheduled to minimize
time-to-first-output-byte and keep the output stream gap-free:
  - recurrence starts at ~6 us: only the first embedding-gather pair
    gates step 0; the other gathers/transposes pipeline into the early
    steps (PE-transpose -> DVE copy straight into the operand buffer),
  - combined fwd+bwd recurrence: step tau = one K=112 matmul + one tanh,
  - at step 78 the recurrence PAUSES and tile 0's softmax-normalizer
    pass (32x [128,1024] exp chunks, ACT-serial ~34 us) runs
    immediately; output streaming starts at ~95 us,
  - position tiles are symmetric pairs {48-79},{32-47,80-95},
    {16-31,96-111},{0-15,112-127}: ready at steps 78/94/110/126, so each
    later tile's 16 recurrence steps + exp pass hide under the previous
    tile's 45.5 us output window,
  - exp partial sums are split across engines (ACT accum_out, Pool
    dummy-copy accum, DVE reduce) to fit ACT's per-window budget,
  - ln(sum) is computed WITHOUT the Ln activation (exponent-bit trick +
    one exp-based Newton step, abs err < 5e-4) so the whole kernel uses
    one activation-table set {Exp,Tanh,Copy,Identity} - no 1.3 us
    table reloads between tanh/exp phases,
  - pass 2 recomputes logit chunks (f32r matmul) and subtracts ln(sum)
    on DVE (ACT shares on the last tile) into staging; output groups
    ramp 1K/1K/2K/2K/4K... columns so the first bytes go out early.
Cost-model exec: ~280 us/core; DMA saturated from ~95 us on.
"""

import numpy as np

import concourse.bacc as bacc
import concourse.tile as tile
from concourse import bass, mybir
from concourse.bass_utils import run_bass_kernel_spmd
from concourse.masks import make_identity
from concourse.tile_rust import add_dep_helper

L = 128
B = 32
V = 32000
EMB = 80
HID = 20
KDIM = EMB + HID          # 100
# Device-side contraction layout: hidden rows at partitions 0:20, zero pad
# 20:32 (compute-engine APs must start 32-aligned), embeddings at 32:112.
EOFF = 32
KP = EOFF + EMB           # 112
H2 = 2 * HID              # 40
NCORES = 8
BL = B // NCORES          # 4 batch columns per core
R = L * BL                # 512 output rows per core
NT = 4                    # position tiles of 128 rows (32 positions)

CH = 1024                 # vocab chunk per pass-1 PSUM tile (2 banks)
NFULL = V // CH           # 31 full chunks
REM = V - NFULL * CH      # 256
NVC = NFULL + 1           # 32 chunks
P2W = 512                 # pass-2 chunk width (1 PSUM bank)
NP2 = (V + P2W - 1) // P2W  # 63 pass-2 chunks (last = 256)
# Output staging groups per tile, in pass-2 chunk counts: ramp up so the
# first bytes hit HBM quickly after the normalizer lands.
GROUP_CHUNKS = [2, 2, 4, 4, 8, 8, 8, 8, 8, 8, 3]
assert sum(GROUP_CHUNKS) == NP2
SGW = 4096                # max staging width (16 KB/partition)

def red_for(pt, vc):
    """Partial-sum engine routing per pass-1 chunk. Even chunks go to
    Pool (reading the SBUF scratch ring), odd to ACT accum_out; the
    serial tile-0 phase also uses the otherwise-idle DVE. The last two
    chunks stay on ACT so the stats gate is just the final exp."""
    if pt == 0:
        if vc % 2 == 0 and vc < 30:
            return "pool"
        if vc % 2 == 1 and vc < 22:
            return "dve"
        return "act"
    if vc % 2 == 0 and vc < 30:
        return "pool"
    return "act"

# ln-approx constants: ln(x) ~= K1*float(bits(x)) - K2, |err| <= 0.0299,
# then one Newton step y += x*exp(-y) - 1 brings |err| < 5e-4.
LN_K1 = 8.262958405176314e-08   # ln2 / 2^23
LN_K2 = 87.99984328235631       # 127*ln2 - 0.02985

# Emission-order deadlines (us since window start) used to merge the
# per-window instruction streams; only relative order matters.
DL_REC = 0.644            # recurrence step period
DL_P2 = 0.658             # DVE subtract pace per 512 chunk
DL_P1_START = 10.9        # first exp after the 16-step tanh batch
DL_P1 = 1.03              # ACT exp pace per 1024 chunk

F32 = mybir.dt.float32
F32R = mybir.dt.float32r
I32 = mybir.dt.int32
AF = mybir.ActivationFunctionType
ALU = mybir.AluOpType
AXL = mybir.AxisListType

_CACHE = {}

# Optional extra kwargs for run_bass_kernel_spmd (used by test harness for
# tracing); harmless defaults for grading.
RUN_KWARGS = {}
LAST_RESULTS = None

# Symmetric position-tile pairs: tile pt = positions [a, a+16) u [b, b+16),
# ready after recurrence step max(b+15, 127-a) = 78/94/110/126.
PTS = [(48, 64), (32, 80), (16, 96), (0, 112)]


def _build():
    nc = bacc.Bacc("TRN2", debug=False, num_devices=NCORES)

    # idx rows 0..511: tokens in (position, batch) row-major order;
    # rows 512..1023: same with positions reversed (backward-chain gather).
    idx_d = nc.dram_tensor("idx", [2 * R, 1], I32, kind="ExternalInput")
    we_d = nc.dram_tensor("we", [V, EMB], F32, kind="ExternalInput")
    i2h_d = nc.dram_tensor("i2h", [KP, HID], F32, kind="ExternalInput")
    # float32r: PE streams fp32 at full rate with tf32-like operand
    # truncation - ~2e-4 relative effect on logits, far inside tolerance.
    h2o_d = nc.dram_tensor("h2o", [H2, V], F32R, kind="ExternalInput")
    biasc_d = nc.dram_tensor("biasc", [HID, 1], F32, kind="ExternalInput")
    h0r_d = nc.dram_tensor("h0r", [HID, 2 * BL], F32, kind="ExternalInput")
    out_d = nc.dram_tensor("out", [R, V], F32, kind="ExternalOutput")

    with tile.TileContext(nc) as tc:
        with (
            tc.tile_pool(name="const", bufs=1) as const,
            tc.tile_pool(name="hbuf", bufs=1) as hbuf,
            tc.tile_pool(name="gat", bufs=2) as gat,
            tc.tile_pool(name="stat", bufs=1) as stat,
            tc.tile_pool(name="stage", bufs=3) as stage,
            tc.tile_pool(name="scr", bufs=4) as scr,
        ):
            ident = const.tile([128, 128], F32)
            make_identity(nc, ident[:])
            # Warm the single activation table ({Exp,Tanh,Copy,Identity}
            # all live in exp_and_others) before the recurrence starts.
            warm = const.tile([1, 1], F32)
            nc.vector.memset(warm[:], 0.0)
            nc.scalar.activation(out=warm[:], in_=warm[:], func=AF.Exp)

            # One strided DMA loads all eight gather index columns (4 fwd +
            # 4 bwd): idx8[p, k] = idx[128k + p].
            idx8 = const.tile([128, 2 * NT], I32)
            nc.sync.dma_start(
                out=idx8[:],
                in_=bass.AP(tensor=idx_d, offset=0, ap=[[1, 128], [128, 2 * NT]]),
            )
            i2h_sb = const.tile([KP, HID], F32)
            nc.sync.dma_start(out=i2h_sb[:], in_=i2h_d[:, :])
            biasc = const.tile([HID, 1], F32)
            nc.sync.dma_start(out=biasc[:], in_=biasc_d[:, :])
            pool_scr = const.tile([128, CH], F32)

            # Combined recurrence operand buffers: step tau's block (8 cols)
            # at tile tau//32, local cols 8*(tau%32): [fwd tau | bwd 127-tau].
            # Rows 0:20 = hidden inputs ([hiddenf[tau] | hiddenb[128-tau]]),
            # rows 20:32 zero pad, rows 32:112 = embedding^T.
            rhsC = [
                hbuf.tile([KP, 256], F32, name=f"rhsC{k}", tag=f"rhsC{k}")
                for k in range(NT)
            ]
            for k in range(NT):
                nc.vector.memset(rhsC[k][HID:EOFF, :], 0.0)
            nc.sync.dma_start(out=rhsC[0][0:HID, 0:8], in_=h0r_d[:, :])

            h2o_sb = const.tile([H2, V], F32R)
            for q in range(4):
                nc.sync.dma_start(
                    out=h2o_sb[:, q * (V // 4) : (q + 1) * (V // 4)],
                    in_=h2o_d[:, q * (V // 4) : (q + 1) * (V // 4)],
                )

            # Embedding gathers, all queued upfront on the gpsimd (SWDGE)
            # queue; transposes/copies pipeline into the recurrence below.
            embG = []
            for k in range(NT):
                for half, icol in ((0, k), (1, NT + k)):
                    g = gat.tile([128, EMB], F32, tag="embG")
                    nc.gpsimd.indirect_dma_start(
                        out=g[:],
                        out_offset=None,
                        in_=we_d[:, :],
                        in_offset=bass.IndirectOffsetOnAxis(
                            ap=idx8[:, icol : icol + 1], axis=0
                        ),
                    )
                    embG.append((k, half, g))

            hcatT = [
                hbuf.tile([H2, 128], F32, name=f"hcatT{k}", tag=f"hcatT{k}")
                for k in range(NT)
            ]
            sparts = [
                stat.tile([128, NVC], F32, name=f"sparts{k}", tag=f"sparts{k}")
                for k in range(NT)
            ]
            logs = [
                stat.tile([128, 1], F32, name=f"logs{k}", tag=f"logs{k}")
                for k in range(NT)
            ]
            neg_log3 = stat.tile([128, 1], F32, name="nlog3", tag="nlog3")

            with (
                tc.tile_pool(name="rps", bufs=2, space="PSUM") as rps,
                tc.tile_pool(name="p1ps", bufs=2, space="PSUM") as p1ps,
                tc.tile_pool(name="p2ps", bufs=2, space="PSUM") as p2ps,
            ):

                def emit_tc(i):
                    # transpose gather i and scatter into rhsC via DVE
                    k, half, g = embG[i]
                    psT = p1ps.tile([128, CH], F32, tag="p1", name="p1t")
                    nc.tensor.transpose(
                        out=psT[0:EMB, 0:128], in_=g[:], identity=ident[:]
                    )
                    dst = rhsC[k][EOFF:, :].rearrange(
                        "p (b g) -> p b g", g=8
                    )[:, :, 4 * half : 4 * half + 4]
                    nc.vector.tensor_copy(out=dst, in_=psT[0:EMB, 0:128])

                def emit_rec(step):
                    k0, c0 = step // 32, 8 * (step % 32)
                    pc = rps.tile([HID, 2 * BL], F32, tag="rec")
                    nc.tensor.matmul(
                        out=pc[:],
                        lhsT=i2h_sb[:],
                        rhs=rhsC[k0][:, c0 : c0 + 8],
                        start=True,
                        stop=True,
                    )
                    t1 = step + 1
                    k1, c1 = t1 // 32, 8 * (t1 % 32)
                    nc.scalar.activation(
                        out=rhsC[k1][0:HID, c1 : c1 + 8],
                        in_=pc[:],
                        func=AF.Tanh,
                        bias=biasc[:],
                    )

                def emit_hcat(pt, after=None):
                    # assemble [40, 128] hidden-state lhsT on DVE (fwd rows
                    # ascending blocks, bwd rows descending blocks).
                    for s, p0 in enumerate(PTS[pt]):
                        d0 = 64 * s
                        kf, fc0 = p0 // 32, 8 * (p0 % 32)
                        tf = rhsC[kf]
                        src_f = bass.AP(
                            tensor=tf.tensor,
                            offset=tf.offset + fc0,
                            ap=[[tf.ap[0][0], HID], [8, 16], [1, 4]],
                        )
                        cp = nc.vector.tensor_copy(
                            out=hcatT[pt][0:HID, d0 : d0 + 64], in_=src_f
                        )
                        if after is not None:
                            add_dep_helper(
                                cp.ins, after.ins, sync=False,
                                reason="hcat behind subtract stream",
                            )
                            after = None
                        b_hi = 127 - p0
                        kb, bc0 = b_hi // 32, 8 * (b_hi % 32) + 4
                        tb = rhsC[kb]
                        src_b = bass.AP(
                            tensor=tb.tensor,
                            offset=tb.offset + bc0,
                            ap=[[tb.ap[0][0], HID], [-8, 16], [1, 4]],
                        )
                        nc.vector.tensor_copy(
                            out=hcatT[pt][HID:, d0 : d0 + 64], in_=src_b
                        )

                def emit_p1(pt, vc, red):
                    # pass-1 chunk: logits to PSUM, exp, partial sum into
                    # sparts[pt][:, vc] via the routed engine. ACT-accum
                    # chunks exp in place (PSUM freed at exp); Pool/DVE
                    # chunks exp into an SBUF scratch ring so the slow
                    # reducer never gates the PSUM rotation.
                    v0 = vc * CH
                    w = CH if vc < NFULL else REM
                    p1t = p1ps.tile([128, CH], F32, tag="p1", name="p1t")
                    for m in range(0, w, 512):
                        mw = min(512, w - m)
                        nc.tensor.matmul(
                            out=p1t[:, m : m + mw],
                            lhsT=hcatT[pt][:].bitcast(F32R),
                            rhs=h2o_sb[:, v0 + m : v0 + m + mw],
                            start=True,
                            stop=True,
                        )
                    col = sparts[pt][:, vc : vc + 1]
                    if red == "act":
                        nc.scalar.activation(
                            out=p1t[:, :w], in_=p1t[:, :w], func=AF.Exp,
                            accum_out=col,
                        )
                    else:
                        sc = scr.tile([128, CH], F32, tag="scr", name="scrt")
                        nc.scalar.activation(
                            out=sc[:, :w], in_=p1t[:, :w], func=AF.Exp
                        )
                        if red == "dve":
                            nc.vector.tensor_reduce(
                                out=col, in_=sc[:, :w], axis=AXL.X, op=ALU.add
                            )
                        else:  # pool: dummy copy with accumulator
                            nc.gpsimd.tensor_scalar(
                                out=pool_scr[:, :w], in0=sc[:, :w],
                                scalar1=1.0, scalar2=None, op0=ALU.mult,
                                accum_out=col,
                            )

                def emit_stats(pt, after=None):
                    # logs[pt] = ln(sum(sparts[pt])) without the Ln table:
                    # exponent-bit approx + one Newton step via Exp.
                    s_t = stat.tile([128, 1], F32, name=f"s{pt}", tag=f"s{pt}")
                    rd = nc.vector.tensor_reduce(
                        out=s_t[:], in_=sparts[pt][:, :], axis=AXL.X, op=ALU.add
                    )
                    if after is not None:
                        # keep the stats chain behind the current tile's
                        # subtract stream on DVE - the scheduler would
                        # otherwise hoist it (and its blocking wait).
                        add_dep_helper(
                            rd.ins, after.ins, sync=False,
                            reason="stats after subtract stream",
                        )
                    fb = stat.tile([128, 1], F32, name=f"fb{pt}", tag=f"fb{pt}")
                    nc.vector.tensor_copy(out=fb[:], in_=s_t[:].bitcast(I32))
                    y0 = stat.tile([128, 1], F32, name=f"y0{pt}", tag=f"y0{pt}")
                    nc.vector.tensor_scalar(
                        out=y0[:], in0=fb[:], scalar1=LN_K1, scalar2=-LN_K2,
                        op0=ALU.mult, op1=ALU.add,
                    )
                    te = stat.tile([128, 1], F32, name=f"te{pt}", tag=f"te{pt}")
                    nc.scalar.activation(
                        out=te[:], in_=y0[:], func=AF.Exp, scale=-1.0
                    )
                    u = stat.tile([128, 1], F32, name=f"u{pt}", tag=f"u{pt}")
                    nc.vector.tensor_mul(out=u[:], in0=s_t[:], in1=te[:])
                    nc.vector.scalar_tensor_tensor(
                        out=logs[pt][:], in0=y0[:], scalar=-1.0, in1=u[:],
                        op0=ALU.add, op1=ALU.add,
                    )
                    if pt == NT - 1:
                        nc.vector.tensor_scalar(
                            out=neg_log3[:], in0=logs[pt][:], scalar1=-1.0,
                            scalar2=None, op0=ALU.mult,
                        )

                # staging state for the output groups of the current tile
                st = {"stg": None, "off": 0, "g0": 0, "gi": 0}

                def emit_p2(pt, j, share_act=False):
                    v0 = j * P2W
                    w = P2W if j < NP2 - 1 else V - v0
                    p2t = p2ps.tile([128, P2W], F32, tag="p2", name="p2t")
                    nc.tensor.matmul(
                        out=p2t[:, :w],
                        lhsT=hcatT[pt][:].bitcast(F32R),
                        rhs=h2o_sb[:, v0 : v0 + w],
                        start=True,
                        stop=True,
                    )
                    if st["off"] == 0:
                        st["stg"] = stage.tile(
                            [128, SGW], F32, tag="stg", name="stg"
                        )
                        st["g0"] = v0
                    off = st["off"]
                    if share_act:
                        sub = nc.scalar.activation(
                            out=st["stg"][:, off : off + w],
                            in_=p2t[:, :w],
                            func=AF.Identity,
                            bias=neg_log3[:],
                        )
                    else:
                        sub = nc.vector.tensor_scalar(
                            out=st["stg"][:, off : off + w],
                            in0=p2t[:, :w],
                            scalar1=logs[pt][:],
                            scalar2=None,
                            op0=ALU.subtract,
                        )
                    st["sub"] = sub
                    st["off"] = off + w
                    # close the group when its chunk count is reached
                    gend = sum(GROUP_CHUNKS[: st["gi"] + 1])
                    if j + 1 == gend:
                        gw = st["off"]
                        r0a, r0b = 4 * PTS[pt][0], 4 * PTS[pt][1]
                        dst = bass.AP(
                            tensor=out_d,
                            offset=r0a * V + st["g0"],
                            ap=[[(r0b - r0a) * V, 2], [V, 64], [1, gw]],
                        )
                        nc.sync.dma_start(out=dst, in_=st["stg"][:, :gw])
                        st["off"] = 0
                        st["gi"] = (st["gi"] + 1) % len(GROUP_CHUNKS)

                # --- prefix: recurrence steps 0..78, gather chains woven in
                emit_tc(0)
                emit_tc(1)
                for step in range(79):
                    emit_rec(step)
                    if step in (8, 16, 24, 32, 40, 48):
                        emit_tc(2 + (step - 8) // 8)

                # --- tile 0 normalizer phase (recurrence paused)
                emit_hcat(0)
                for vc in range(NVC):
                    emit_p1(0, vc, red_for(0, vc))
                emit_stats(0)

                # --- four output windows; window pt streams tile pt while
                # the next tile's recurrence tail + exp pass run under it.
                for pt in range(NT):
                    last = pt == NT - 1
                    ev = []
                    if not last:
                        for i in range(16):
                            ev.append((DL_REC * i, 0, "rec", 79 + 16 * pt + i))
                        for k in range(NVC):
                            ev.append(
                                (DL_P1_START + DL_P1 * k - 0.6, 1, "p1", k)
                            )
                    for j in range(NP2):
                        ev.append((max(DL_P2 * j - 0.4, 0.05), 2, "p2", j))
                    ev.sort(key=lambda e: (e[0], e[1]))
                    for _, _, kind, a in ev:
                        if kind == "rec":
                            emit_rec(a)
                        elif kind == "p1":
                            emit_p1(pt + 1, a, red_for(pt + 1, a))
                        else:
                            emit_p2(pt, a, share_act=last and (a % 2 == 1))
                            if a == 12 and not last:
                                # assemble the next tile's hidden-state
                                # lhsT now: behind ~13 subtracts (so the
                                # copies' tanh wait can't stall the
                                # output stream) but before the first
                                # pass-1 matmul needs it.
                                emit_hcat(pt + 1, after=st["sub"])
                    if not last:
                        emit_stats(pt + 1, after=st["sub"])

    nc.compile()
    return nc


def _get_nc():
    if "nc" not in _CACHE:
        _CACHE["nc"] = _build()
    return _CACHE["nc"]


def kernel(input, we, i2h, h2o, bias, h0):
    global LAST_RESULTS
    input = np.asarray(input)
    we = np.ascontiguousarray(np.asarray(we), dtype=np.float32)
    i2h = np.ascontiguousarray(np.asarray(i2h), dtype=np.float32)
    h2o = np.ascontiguousarray(np.asarray(h2o), dtype=np.float32)
    bias = np.asarray(bias, dtype=np.float32)
    h0 = np.asarray(h0, dtype=np.float32)

    biasc = np.ascontiguousarray(bias.reshape(1, HID).T)          # [20, 1]
    h0r = np.ascontiguousarray(
        np.repeat(h0.reshape(1, HID).T, 2 * BL, axis=1)           # [20, 8]
    )
    # Reorder i2h into the padded device contraction layout: hidden-state
    # weight rows first, zeros, then embedding weight rows.
    i2h_dev = np.zeros((KP, HID), dtype=np.float32)
    i2h_dev[0:HID] = i2h[EMB:]
    i2h_dev[EOFF:] = i2h[0:EMB]

    nc = _get_nc()
    in_maps = []
    for c in range(NCORES):
        tok = input[:, BL * c : BL * (c + 1)].astype(np.int32)    # [L, BL]
        idx = np.ascontiguousarray(
            np.concatenate([tok.reshape(R), tok[::-1].reshape(R)]).reshape(
                2 * R, 1
            )
        )
        in_maps.append(
            {
                "idx": idx,
                "we": we,
                "i2h": i2h_dev,
                "h2o": h2o,
                "biasc": biasc,
                "h0r": h0r,
            }
        )

    res = run_bass_kernel_spmd(
        nc, in_maps, core_ids=list(range(NCORES)), **RUN_KWARGS
    )
    LAST_RESULTS = res
    parts = [res.results[c]["out"].reshape(L, BL, V) for c in range(NCORES)]
    return np.concatenate(parts, axis=1)


# revision 15
# speedup vs baseline: 1.3913x; 1.2712x over previous
"""Trainium2 Bass kernel for the bidirectional RNN language model.

Model (see problem reference): for a [L=128, B=32] int token grid,
  - forward + backward tanh-RNN (HID=20) over EMB=80 embeddings (VOCAB=32000)
  - per position: logits = [h_fwd[i], h_bwd[i+1]] @ h2o   -> [*, 32000]
  - output log_softmax(logits)  ->  [128, 32, 32000] f32  (512 MB)

Strategy: data-parallel over batch across 8 NeuronCores (4 batch columns
per core), no collectives. The 64 MB/core output write (~185 us at DMA
roofline) is the hard floor; everything else is scheduled to minimize
time-to-first-output-byte and keep the output stream gap-free:
  - recurrence starts at ~6 us: only the first embedding-gather pair
    gates step 0; the other gathers/transposes pipeline into the early
    steps (PE-transpose -> DVE copy straight into the operand buffer),
  - combined fwd+bwd recurrence: step tau = one K=112 matmul + one tanh,
  - at step 78 the recurrence PAUSES and tile 0's softmax-normalizer
    pass (32x [128,1024] exp chunks, ACT-serial ~34 us) runs
    immediately; output streaming starts at ~95 us,
  - position tiles are symmetric pairs {48-79},{32-47,80-95},
    {16-31,96-111},{0-15,112-127}: ready at steps 78/94/110/126, so each
    later tile's 16 recurrence steps + exp pass hide under the previous
    tile's 45.5 us output window,
  - exp partial sums are split across engines (ACT accum_out, Pool
    dummy-copy accum, DVE reduce) to fit ACT's per-window budget,
  - ln(sum) is computed WITHOUT the Ln activation (exponent-bit trick +
    one exp-based Newton step, abs err < 5e-4) so the whole kernel uses
    one activation-table set {Exp,Tanh,Copy,Identity} - no 1.3 us
    table reloads between tanh/exp phases,
  - pass 2 recomputes logit chunks (f32r matmul) and subtracts ln(sum)
    on DVE (ACT shares on the last tile) into staging; output groups
    ramp 1K/1K/2K/2K/4K... columns so the first bytes go out early.
Cost-model exec: ~280 us/core; DMA saturated from ~95 us on.
"""

import numpy as np

import concourse.bacc as bacc
import concourse.tile as tile
from concourse import bass, mybir
from concourse.bass_utils import run_bass_kernel_spmd
from concourse.masks import make_identity
from concourse.tile_rust import add_dep_helper

L = 128
B = 32
V = 32000
EMB = 80
HID = 20
KDIM = EMB + HID          # 100
# Device-side contraction layout: hidden rows at partitions 0:20, zero pad
# 20:32 (compute-engine APs must start 32-aligned), embeddings at 32:112.
EOFF = 32
KP = EOFF + EMB           # 112
H2 = 2 * HID              # 40
NCORES = 8
BL = B // NCORES          # 4 batch columns per core
R = L * BL                # 512 output rows per core
NT = 4                    # position tiles of 128 rows (32 positions)

CH = 1024                 # vocab chunk per pass-1 PSUM tile (2 banks)
NFULL = V // CH           # 31 full chunks
REM = V - NFULL * CH      # 256
NVC = NFULL + 1           # 32 chunks
P2W = 512                 # pass-2 chunk width (1 PSUM bank)
NP2 = (V + P2W - 1) // P2W  # 63 pass-2 chunks (last = 256)
# Output staging groups per tile, in pass-2 chunk counts: ramp up so the
# first bytes hit HBM quickly after the normalizer lands.
GROUP_CHUNKS = [2, 2, 4, 4, 8, 8, 8, 8, 8, 8, 3]
assert sum(GROUP_CHUNKS) == NP2
SGW = 4096                # max staging width (16 KB/partition)

def red_for(pt, vc):
    """Partial-sum engine routing per pass-1 chunk. Even chunks go to
    Pool (reading the SBUF scratch ring), odd to ACT accum_out; the
    serial tile-0 phase also uses the otherwise-idle DVE. The last two
    chunks stay on ACT so the stats gate is just the final exp."""
    if pt == 0:
        if vc % 2 == 0 and vc < 30:
            return "pool"
        if vc % 2 == 1 and vc < 22:
            return "dve"
        return "act"
    if vc % 2 == 0 and vc < 30:
        return "pool"
    return "act"

# ln-approx constants: ln(x) ~= K1*float(bits(x)) - K2, |err| <= 0.0299,
# then one Newton step y += x*exp(-y) - 1 brings |err| < 5e-4.
LN_K1 = 8.262958405176314e-08   # ln2 / 2^23
LN_K2 = 87.99984328235631       # 127*ln2 - 0.02985

# Emission-order deadlines (us since window start) used to merge the
# per-window instruction streams; only relative order matters.
DL_REC = 0.644            # recurrence step period
DL_P2 = 0.658             # DVE subtract pace per 512 chunk
DL_P1_START = 10.9        # first exp after the 16-step tanh batch
DL_P1 = 1.03              # ACT exp pace per 1024 chunk

F32 = mybir.dt.float32
F32R = mybir.dt.float32r
I32 = mybir.dt.int32
AF = mybir.ActivationFunctionType
ALU = mybir.AluOpType
AXL = mybir.AxisListType

_CACHE = {}

# Optional extra kwargs for run_bass_kernel_spmd (used by test harness for
# tracing); harmless defaults for grading.
RUN_KWARGS = {}
LAST_RESULTS = None

# Symmetric position-tile pairs: tile pt = positions [a, a+16) u [b, b+16),
# ready after recurrence step max(b+15, 127-a) = 78/94/110/126.
PTS = [(48, 64), (32, 80), (16, 96), (0, 112)]


def _build():
    nc = bacc.Bacc("TRN2", debug=False, num_devices=NCORES)

    # idx rows 0..511: tokens in (position, batch) row-major order;
    # rows 512..1023: same with positions reversed (backward-chain gather).
    idx_d = nc.dram_tensor("idx", [2 * R, 1], I32, kind="ExternalInput")
    we_d = nc.dram_tensor("we", [V, EMB], F32, kind="ExternalInput")
    i2h_d = nc.dram_tensor("i2h", [KP, HID], F32, kind="ExternalInput")
    # float32r: PE streams fp32 at full rate with tf32-like operand
    # truncation - ~2e-4 relative effect on logits, far inside tolerance.
    h2o_d = nc.dram_tensor("h2o", [H2, V], F32R, kind="ExternalInput")
    biasc_d = nc.dram_tensor("biasc", [HID, 1], F32, kind="ExternalInput")
    h0r_d = nc.dram_tensor("h0r", [HID, 2 * BL], F32, kind="ExternalInput")
    out_d = nc.dram_tensor("out", [R, V], F32, kind="ExternalOutput")

    with tile.TileContext(nc) as tc:
        with (
            tc.tile_pool(name="const", bufs=1) as const,
            tc.tile_pool(name="hbuf", bufs=1) as hbuf,
            tc.tile_pool(name="gat", bufs=2) as gat,
            tc.tile_pool(name="stat", bufs=1) as stat,
            tc.tile_pool(name="stage", bufs=3) as stage,
            tc.tile_pool(name="scr", bufs=4) as scr,
        ):
            ident = const.tile([128, 128], F32)
            make_identity(nc, ident[:])
            # Warm the single activation table ({Exp,Tanh,Copy,Identity}
            # all live in exp_and_others) before the recurrence starts.
            warm = const.tile([1, 1], F32)
            nc.vector.memset(warm[:], 0.0)
            nc.scalar.activation(out=warm[:], in_=warm[:], func=AF.Exp)

            # One strided DMA loads all eight gather index columns (4 fwd +
            # 4 bwd): idx8[p, k] = idx[128k + p].
            idx8 = const.tile([128, 2 * NT], I32)
            nc.sync.dma_start(
                out=idx8[:],
                in_=bass.AP(tensor=idx_d, offset=0, ap=[[1, 128], [128, 2 * NT]]),
            )
            i2h_sb = const.tile([KP, HID], F32)
            nc.sync.dma_start(out=i2h_sb[:], in_=i2h_d[:, :])
            biasc = const.tile([HID, 1], F32)
            nc.sync.dma_start(out=biasc[:], in_=biasc_d[:, :])
            pool_scr = const.tile([128, CH], F32)

            # Combined recurrence operand buffers: step tau's block (8 cols)
            # at tile tau//32, local cols 8*(tau%32): [fwd tau | bwd 127-tau].
            # Rows 0:20 = hidden inputs ([hiddenf[tau] | hiddenb[128-tau]]),
            # rows 20:32 zero pad, rows 32:112 = embedding^T.
            rhsC = [
                hbuf.tile([KP, 256], F32, name=f"rhsC{k}", tag=f"rhsC{k}")
                for k in range(NT)
            ]
            for k in range(NT):
                nc.vector.memset(rhsC[k][HID:EOFF, :], 0.0)
            nc.sync.dma_start(out=rhsC[0][0:HID, 0:8], in_=h0r_d[:, :])

            h2o_sb = const.tile([H2, V], F32R)
            for q in range(4):
                nc.sync.dma_start(
                    out=h2o_sb[:, q * (V // 4) : (q + 1) * (V // 4)],
                    in_=h2o_d[:, q * (V // 4) : (q + 1) * (V // 4)],
                )

            # Embedding gathers, all queued upfront on the gpsimd (SWDGE)
            # queue; transposes/copies pipeline into the recurrence below.
            embG = []
            for k in range(NT):
                for half, icol in ((0, k), (1, NT + k)):
                    g = gat.tile([128, EMB], F32, tag="embG")
                    nc.gpsimd.indirect_dma_start(
                        out=g[:],
                        out_offset=None,
                        in_=we_d[:, :],
                        in_offset=bass.IndirectOffsetOnAxis(
                            ap=idx8[:, icol : icol + 1], axis=0
                        ),
                    )
                    embG.append((k, half, g))

            hcatT = [
                hbuf.tile([H2, 128], F32, name=f"hcatT{k}", tag=f"hcatT{k}")
                for k in range(NT)
            ]
            sparts = [
                stat.tile([128, NVC], F32, name=f"sparts{k}", tag=f"sparts{k}")
                for k in range(NT)
            ]
            logs = [
                stat.tile([128, 1], F32, name=f"logs{k}", tag=f"logs{k}")
                for k in range(NT)
            ]
            neg_log3 = stat.tile([128, 1], F32, name="nlog3", tag="nlog3")

            with (
                tc.tile_pool(name="rps", bufs=2, space="PSUM") as rps,
                tc.tile_pool(name="p1ps", bufs=2, space="PSUM") as p1ps,
                tc.tile_pool(name="p2ps", bufs=2, space="PSUM") as p2ps,
            ):

                def emit_tc(i):
                    # transpose gather i and scatter into rhsC via DVE
                    k, half, g = embG[i]
                    psT = p1ps.tile([128, CH], F32, tag="p1", name="p1t")
                    nc.tensor.transpose(
                        out=psT[0:EMB, 0:128], in_=g[:], identity=ident[:]
                    )
                    dst = rhsC[k][EOFF:, :].rearrange(
                        "p (b g) -> p b g", g=8
                    )[:, :, 4 * half : 4 * half + 4]
                    nc.vector.tensor_copy(out=dst, in_=psT[0:EMB, 0:128])

                def emit_rec(step):
                    k0, c0 = step // 32, 8 * (step % 32)
                    pc = rps.tile([HID, 2 * BL], F32, tag="rec")
                    nc.tensor.matmul(
                        out=pc[:],
                        lhsT=i2h_sb[:],
                        rhs=rhsC[k0][:, c0 : c0 + 8],
                        start=True,
                        stop=True,
                    )
                    t1 = step + 1
                    k1, c1 = t1 // 32, 8 * (t1 % 32)
                    nc.scalar.activation(
                        out=rhsC[k1][0:HID, c1 : c1 + 8],
                        in_=pc[:],
                        func=AF.Tanh,
                        bias=biasc[:],
                    )

                def emit_hcat(pt, after=None):
                    # assemble [40, 128] hidden-state lhsT on DVE (fwd rows
                    # ascending blocks, bwd rows descending blocks).
                    for s, p0 in enumerate(PTS[pt]):
                        d0 = 64 * s
                        kf, fc0 = p0 // 32, 8 * (p0 % 32)
                        tf = rhsC[kf]
                        src_f = bass.AP(
                            tensor=tf.tensor,
                            offset=tf.offset + fc0,
                            ap=[[tf.ap[0][0], HID], [8, 16], [1, 4]],
                        )
                        cp = nc.vector.tensor_copy(
                            out=hcatT[pt][0:HID, d0 : d0 + 64], in_=src_f
                        )
                        if after is not None:
                            add_dep_helper(
                                cp.ins, after.ins, sync=False,
                                reason="hcat behind subtract stream",
                            )
                            after = None
                        b_hi = 127 - p0
                        kb, bc0 = b_hi // 32, 8 * (b_hi % 32) + 4
                        tb = rhsC[kb]
                        src_b = bass.AP(
                            tensor=tb.tensor,
                            offset=tb.offset + bc0,
                            ap=[[tb.ap[0][0], HID], [-8, 16], [1, 4]],
                        )
                        nc.vector.tensor_copy(
                            out=hcatT[pt][HID:, d0 : d0 + 64], in_=src_b
                        )

                def emit_p1(pt, vc, red, after=None):
                    # pass-1 chunk: logits to PSUM, exp, partial sum into
                    # sparts[pt][:, vc] via the routed engine. ACT-accum
                    # chunks exp in place (PSUM freed at exp); Pool/DVE
                    # chunks exp into an SBUF scratch ring so the slow
                    # reducer never gates the PSUM rotation.
                    v0 = vc * CH
                    w = CH if vc < NFULL else REM
                    p1t = p1ps.tile([128, CH], F32, tag="p1", name="p1t")
                    for m in range(0, w, 512):
                        mw = min(512, w - m)
                        mm = nc.tensor.matmul(
                            out=p1t[:, m : m + mw],
                            lhsT=hcatT[pt][:].bitcast(F32R),
                            rhs=h2o_sb[:, v0 + m : v0 + m + mw],
                            start=True,
                            stop=True,
                        )
                        if after is not None:
                            # Pin PE order: keep this pass-1 matmul behind
                            # the paired pass-2 matmul so the scheduler
                            # can't starve the DVE/DMA stream by hoisting
                            # P1 work.
                            add_dep_helper(
                                mm.ins, after.ins, sync=False,
                                reason="wave interleave order",
                            )
                            after = None
                    col = sparts[pt][:, vc : vc + 1]
                    if red == "act":
                        nc.scalar.activation(
                            out=p1t[:, :w], in_=p1t[:, :w], func=AF.Exp,
                            accum_out=col,
                        )
                    else:
                        sc = scr.tile([128, CH], F32, tag="scr", name="scrt")
                        nc.scalar.activation(
                            out=sc[:, :w], in_=p1t[:, :w], func=AF.Exp
                        )
                        if red == "dve":
                            nc.vector.tensor_reduce(
                                out=col, in_=sc[:, :w], axis=AXL.X, op=ALU.add
                            )
                        else:  # pool: dummy copy with accumulator
                            nc.gpsimd.tensor_scalar(
                                out=pool_scr[:, :w], in0=sc[:, :w],
                                scalar1=1.0, scalar2=None, op0=ALU.mult,
                                accum_out=col,
                            )

                def emit_stats(pt, after=None):
                    # logs[pt] = ln(sum(sparts[pt])) without the Ln table:
                    # exponent-bit approx + one Newton step via Exp.
                    s_t = stat.tile([128, 1], F32, name=f"s{pt}", tag=f"s{pt}")
                    rd = nc.vector.tensor_reduce(
                        out=s_t[:], in_=sparts[pt][:, :], axis=AXL.X, op=ALU.add
                    )
                    if after is not None:
                        # keep the stats chain behind the current tile's
                        # subtract stream on DVE - the scheduler would
                        # otherwise hoist it (and its blocking wait).
                        add_dep_helper(
                            rd.ins, after.ins, sync=False,
                            reason="stats after subtract stream",
                        )
                    fb = stat.tile([128, 1], F32, name=f"fb{pt}", tag=f"fb{pt}")
                    nc.vector.tensor_copy(out=fb[:], in_=s_t[:].bitcast(I32))
                    y0 = stat.tile([128, 1], F32, name=f"y0{pt}", tag=f"y0{pt}")
                    nc.vector.tensor_scalar(
                        out=y0[:], in0=fb[:], scalar1=LN_K1, scalar2=-LN_K2,
                        op0=ALU.mult, op1=ALU.add,
                    )
                    te = stat.tile([128, 1], F32, name=f"te{pt}", tag=f"te{pt}")
                    nc.scalar.activation(
                        out=te[:], in_=y0[:], func=AF.Exp, scale=-1.0
                    )
                    u = stat.tile([128, 1], F32, name=f"u{pt}", tag=f"u{pt}")
                    nc.vector.tensor_mul(out=u[:], in0=s_t[:], in1=te[:])
                    nc.vector.scalar_tensor_tensor(
                        out=logs[pt][:], in0=y0[:], scalar=-1.0, in1=u[:],
                        op0=ALU.add, op1=ALU.add,
                    )
                    if pt == NT - 1:
                        nc.vector.tensor_scalar(
                            out=neg_log3[:], in0=logs[pt][:], scalar1=-1.0,
                            scalar2=None, op0=ALU.mult,
                        )

                # staging state for the output groups of the current tile
                st = {"stg": None, "off": 0, "g0": 0, "gi": 0}

                def emit_p2(pt, j, share_act=False):
                    v0 = j * P2W
                    w = P2W if j < NP2 - 1 else V - v0
                    p2t = p2ps.tile([128, P2W], F32, tag="p2", name="p2t")
                    st["mm"] = nc.tensor.matmul(
                        out=p2t[:, :w],
                        lhsT=hcatT[pt][:].bitcast(F32R),
                        rhs=h2o_sb[:, v0 : v0 + w],
                        start=True,
                        stop=True,
                    )
                    if st["off"] == 0:
                        st["stg"] = stage.tile(
                            [128, SGW], F32, tag="stg", name="stg"
                        )
                        st["g0"] = v0
                    off = st["off"]
                    if share_act:
                        sub = nc.scalar.activation(
                            out=st["stg"][:, off : off + w],
                            in_=p2t[:, :w],
                            func=AF.Identity,
                            bias=neg_log3[:],
                        )
                    else:
                        sub = nc.vector.tensor_scalar(
                            out=st["stg"][:, off : off + w],
                            in0=p2t[:, :w],
                            scalar1=logs[pt][:],
                            scalar2=None,
                            op0=ALU.subtract,
                        )
                    st["sub"] = sub
                    st["off"] = off + w
                    # close the group when its chunk count is reached
                    gend = sum(GROUP_CHUNKS[: st["gi"] + 1])
                    if j + 1 == gend:
                        gw = st["off"]
                        r0a, r0b = 4 * PTS[pt][0], 4 * PTS[pt][1]
                        dst = bass.AP(
                            tensor=out_d,
                            offset=r0a * V + st["g0"],
                            ap=[[(r0b - r0a) * V, 2], [V, 64], [1, gw]],
                        )
                        nc.sync.dma_start(out=dst, in_=st["stg"][:, :gw])
                        st["off"] = 0
                        st["gi"] = (st["gi"] + 1) % len(GROUP_CHUNKS)

                # --- prefix: recurrence steps 0..78, gather chains woven in
                emit_tc(0)
                emit_tc(1)
                for step in range(79):
                    emit_rec(step)
                    if step in (8, 16, 24, 32, 40, 48):
                        emit_tc(2 + (step - 8) // 8)

                # --- tile 0 normalizer phase (recurrence paused)
                emit_hcat(0)
                for vc in range(NVC):
                    emit_p1(0, vc, red_for(0, vc))
                emit_stats(0)

                # --- four output windows; window pt streams tile pt while
                # the next tile's recurrence tail + exp pass run under it.
                for pt in range(NT):
                    last = pt == NT - 1
                    ev = []
                    if not last:
                        for i in range(16):
                            ev.append((DL_REC * i, 0, "rec", 79 + 16 * pt + i))
                        for k in range(NVC):
                            ev.append(
                                (DL_P1_START + DL_P1 * k - 0.6, 1, "p1", k)
                            )
                    for j in range(NP2):
                        ev.append((max(DL_P2 * j - 0.4, 0.05), 2, "p2", j))
                    ev.sort(key=lambda e: (e[0], e[1]))
                    for _, _, kind, a in ev:
                        if kind == "rec":
                            emit_rec(a)
                        elif kind == "p1":
                            emit_p1(pt + 1, a, red_for(pt + 1, a),
                                    after=st.get("mm"))
                        else:
                            emit_p2(pt, a, share_act=last and (a % 2 == 1))
                            if a == 12 and not last:
                                # assemble the next tile's hidden-state
                                # lhsT now: behind ~13 subtracts (so the
                                # copies' tanh wait can't stall the
                                # output stream) but before the first
                                # pass-1 matmul needs it.
                                emit_hcat(pt + 1, after=st["sub"])
                    if not last:
                        emit_stats(pt + 1, after=st["sub"])

    nc.compile()
    return nc


def _get_nc():
    if "nc" not in _CACHE:
        _CACHE["nc"] = _build()
    return _CACHE["nc"]


def kernel(input, we, i2h, h2o, bias, h0):
    global LAST_RESULTS
    input = np.asarray(input)
    we = np.ascontiguousarray(np.asarray(we), dtype=np.float32)
    i2h = np.ascontiguousarray(np.asarray(i2h), dtype=np.float32)
    h2o = np.ascontiguousarray(np.asarray(h2o), dtype=np.float32)
    bias = np.asarray(bias, dtype=np.float32)
    h0 = np.asarray(h0, dtype=np.float32)

    biasc = np.ascontiguousarray(bias.reshape(1, HID).T)          # [20, 1]
    h0r = np.ascontiguousarray(
        np.repeat(h0.reshape(1, HID).T, 2 * BL, axis=1)           # [20, 8]
    )
    # Reorder i2h into the padded device contraction layout: hidden-state
    # weight rows first, zeros, then embedding weight rows.
    i2h_dev = np.zeros((KP, HID), dtype=np.float32)
    i2h_dev[0:HID] = i2h[EMB:]
    i2h_dev[EOFF:] = i2h[0:EMB]

    nc = _get_nc()
    in_maps = []
    for c in range(NCORES):
        tok = input[:, BL * c : BL * (c + 1)].astype(np.int32)    # [L, BL]
        idx = np.ascontiguousarray(
            np.concatenate([tok.reshape(R), tok[::-1].reshape(R)]).reshape(
                2 * R, 1
            )
        )
        in_maps.append(
            {
                "idx": idx,
                "we": we,
                "i2h": i2h_dev,
                "h2o": h2o,
                "biasc": biasc,
                "h0r": h0r,
            }
        )

    res = run_bass_kernel_spmd(
        nc, in_maps, core_ids=list(range(NCORES)), **RUN_KWARGS
    )
    LAST_RESULTS = res
    parts = [res.results[c]["out"].reshape(L, BL, V) for c in range(NCORES)]
    return np.concatenate(parts, axis=1)
